# revision 11
# baseline (speedup 1.0000x reference)
"""Correlation (FlowNet-style, max_displacement=4) on 8 TRN2 NeuronCores.

Full inputs x1, x2: [B=8, C=64, H=192, W=192] fp32. Output: [8, 81, 192, 192] fp32.
out[b, di*9+dj, h, w] = mean_c x1[b,c,h,w] * x2pad[b,c,h+di,w+dj]   (di,dj in [0,9))

Strategy: batch-parallel (1 batch per core). Per core the correlation is computed
as a banded Gram matrix on the TensorEngine: for each 16x8 (h,w) output tile,
one bf16 matmul with lhsT = x1 tile [K=64 channels, M=128 pixels] and rhs = padded
x2 window [64, 24*16=384] produces all 81 displacement dot products of every tile
pixel inside a skewed band of the 128x384 PSUM result. PSUM is evicted
(fp32->bf16) to SBUF by DVE/ACT in two-tile ops. The band is shipped in
4-dh-group blocks: partitions [32b, 32b+32) x band-column union [64b, 64b+192)
x both halves — 64 descriptors of 9216 B per DMA, which sprays across all 16
SDMA engines (8/16-descriptor DMAs only ever land on engines 0-7) at good
per-descriptor efficiency, for 1.33x byte inflation over the exact band
parallelogram (2.37x the useful output vs 4.74x if the whole PSUM band were
shipped). Only 4 out-DMAs per strip (24 total), alternating between the two
HWDGE rings (sync/scalar), so sequencer descriptor-gen (~640+40*ndesc ns per
DMA) stays off the critical path. The band is deskewed on the host with a
zero-copy strided view. x1 is pre-scaled by 1/64 on the host (exact) so the
matmul output is directly the channel mean.

The h axis is split into two halves living on partitions 0-63 / 64-127 (K=64
each), interleaved so paired matmuls run concurrently on disjoint PE
row-groups. Inputs are loaded in three h-chunks (separate tiles, small first
chunk) interleaved with compute so the PE starts early.
"""

import sys
import types

import numpy as np
import ml_dtypes

import concourse.bacc as bacc
from concourse import mybir
from concourse.tile import TileContext
from concourse.bass_utils import run_bass_kernel_spmd

B, C, H, W = 8, 64, 192, 192
MAXD = 4
D = 2 * MAXD + 1  # 9
HP, WP = H + 2 * MAXD, W + 2 * MAXD  # 200, 200

TH, TW = 16, 8            # output tile (h, w) -> M = 128
NH, NW = TH + 2 * MAXD, TW + 2 * MAXD  # x2 window 24 x 16 -> N = 384
NSP = H // (2 * TH)       # 6 strips per partition-half
N_WT = W // TW            # 24 w-tiles
HHALF = H // 2            # 96 rows per partition-half
SLAB = HHALF + 2 * MAXD   # 104 padded x2 rows per half
BCOL = D * NW             # 144 band columns per dh-group
GB = 4                    # dh-groups per out-DMA block
NB = TH // GB             # 4 blocks per strip
BW = BCOL + (GB - 1) * NW  # 192 block band columns

# Input h-chunking: strip ranges per chunk and the x2 slab rows they need.
X1_CHUNKS = [(0, 1), (1, 3), (3, 6)]              # strip ranges
X2_CHUNKS = [(0, 24), (16, 56), (48, 104)]        # x2 local row ranges

BF16 = ml_dtypes.bfloat16


def _install_axon_trace_shim():
    """The image's antenv package lacks axon_hooks; run_bass_kernel_spmd
    crashes on import when trace=True. Provide the hook from the boot module
    so tracing works instead of raising."""
    if "antenv.axon_hooks" in sys.modules:
        return
    try:
        import trn_agent_boot.trn_boot as tb

        hook = tb._ntff_profile_via_ctypes("/opt/axon/libaxon_pjrt.so")
    except Exception:
        hook = None
    mod = types.ModuleType("antenv.axon_hooks")
    mod.get_axon_ntff_profile_hook = lambda: hook
    mod.set_axon_ntff_profile_hook = lambda h: None
    sys.modules["antenv.axon_hooks"] = mod


def build_nc():
    nc = bacc.Bacc("TRN2", target_bir_lowering=False, debug=False)
    # x1 arrives pre-tiled: [128, strip, wtile, 128 pixels] — walrus requires
    # the matmul weights AP to have a single free dimension.
    x1s = nc.dram_tensor("x1s", [128, NSP, N_WT, TH * TW], mybir.dt.bfloat16, kind="ExternalInput")
    x2s = nc.dram_tensor("x2s", [128, SLAB, WP], mybir.dt.bfloat16, kind="ExternalInput")
    y = nc.dram_tensor("y", [NSP, NB, GB * TW, 2, BW, N_WT], mybir.dt.bfloat16, kind="ExternalOutput")

    with TileContext(nc) as tc:
        with (
            tc.tile_pool(name="imgs", bufs=1) as imgs,
            tc.tile_pool(name="outs", bufs=3) as outs,
            tc.tile_pool(name="psum", bufs=4, space="PSUM") as psum,
        ):
            # Chunked input tiles (separate tiles -> precise chunk->matmul deps).
            # Chunk 0 loads go through SWDGE (gpsimd): descriptor gen is
            # SPMD across the 8 Q7 cores instead of ~5.8us serial on a HWDGE
            # sequencer (640+40*ndesc ns), pulling in the first matmul.
            x1c, x2c = [], []
            for ci in range(3):
                s0, s1 = X1_CHUNKS[ci]
                r0, r1 = X2_CHUNKS[ci]
                x2t = imgs.tile([128, r1 - r0, WP], mybir.dt.bfloat16, tag=f"x2c{ci}")
                x2_eng = nc.gpsimd if ci == 0 else nc.sync
                x2_eng.dma_start(out=x2t[:], in_=x2s[:, r0:r1, :])
                x1t = imgs.tile([128, s1 - s0, N_WT, TH * TW], mybir.dt.bfloat16, tag=f"x1c{ci}")
                x1_eng = nc.gpsimd if ci == 0 else nc.scalar
                x1_eng.dma_start(out=x1t[:], in_=x1s[:, s0:s1])
                x2c.append(x2t)
                x1c.append(x1t)

            copy_k = 0
            for sp in range(NSP):
                ci = next(i for i, (s0, s1) in enumerate(X1_CHUNKS) if s0 <= sp < s1)
                hl = sp * TH - X2_CHUNKS[ci][0]   # row offset within x2 chunk
                spl = sp - X1_CHUNKS[ci][0]       # strip offset within x1 chunk
                # Both halves in one tile; [col, t] minor so each band block
                # is one contiguous run per (partition, half).
                ybuf = outs.tile([128, 2, NH * NW, N_WT], mybir.dt.bfloat16,
                                 name=f"ybuf_{sp}", tag="ybuf")
                for tp in range(N_WT // 2):       # pairs of w-tiles
                    # Interleave the two partition halves so adjacent matmuls
                    # sit on disjoint PE row-groups and execute concurrently.
                    for half in range(2):
                        p0 = 64 * half
                        pt = psum.tile([128, 1024], mybir.dt.float32)
                        for u in range(2):
                            t = 2 * tp + u
                            w0 = t * TW
                            nc.tensor.matmul(
                                pt[:, 512 * u:512 * u + NH * NW],
                                lhsT=x1c[ci][p0:p0 + 64, spl, t, :],
                                rhs=x2c[ci][p0:p0 + 64, hl:hl + NH, w0:w0 + NW],
                                start=True, stop=True,
                            )
                        # Evict both tiles with one op; alternate DVE / ACT.
                        src = pt[:].rearrange("p (a b) -> p b a", a=2)[:, 0:NH * NW, :]
                        dst = ybuf[:, half, :, 2 * tp:2 * tp + 2]
                        if copy_k % 2 == 0:
                            nc.vector.tensor_copy(dst, src)
                        else:
                            nc.scalar.copy(dst, src)
                        copy_k += 1
                # Band out in 4-group blocks: partitions [32b, 32b+32), column
                # union [64b, 64b+192), both halves: 64 descriptors x 9216 B.
                for blk in range(NB):
                    eng = nc.sync if (sp + blk) % 2 == 0 else nc.scalar
                    eng.dma_start(
                        out=y[sp, blk],
                        in_=ybuf[32 * blk:32 * blk + 32, :,
                                 NW * GB * blk:NW * GB * blk + BW, :],
                    )

    nc.compile()
    return nc


_NC_CACHE = None


def _get_nc():
    global _NC_CACHE
    if _NC_CACHE is None:
        _NC_CACHE = build_nc()
    return _NC_CACHE


def _prep_inputs(x1, x2):
    """Host-side shard prep: scale, pad, split h into partition halves, bf16."""
    in_maps = []
    x1 = np.asarray(x1, dtype=np.float32)
    x2 = np.asarray(x2, dtype=np.float32)
    x1h = (x1 * (1.0 / C)).astype(BF16)
    x2h = x2.astype(BF16)
    for b in range(B):
        # x1: [64, 192, 192] -> pre-tiled [128 = half*64+c, sp, t, dh*TW+dw]
        a = x1h[b].reshape(C, 2, NSP, TH, N_WT, TW)
        a = a.transpose(1, 0, 2, 4, 3, 5).reshape(128, NSP, N_WT, TH * TW)
        # x2: pad to [64, 200, 200], two overlapping 104-row slabs
        p = np.zeros((C, HP, WP), dtype=BF16)
        p[:, MAXD:MAXD + H, MAXD:MAXD + W] = x2h[b]
        s = np.stack([p[:, 0:SLAB, :], p[:, HHALF:HHALF + SLAB, :]], axis=0)
        s = s.reshape(2 * C, SLAB, WP)
        in_maps.append({"x1s": np.ascontiguousarray(a), "x2s": np.ascontiguousarray(s)})
    return in_maps


def _deskew(yb):
    """yb: [NSP, NB, GB*TW, 2, BW, N_WT] fp32 (one batch) -> [81, 192, 192].

    h = half*96 + sp*TH + 4*blk + gin,  w = t*TW + dw; the value for
    displacement (di, dj) at (gin, dw) sits at block column
    16*gin + 16*di + dw + dj of partition gin*8+dw.
    """
    s_sp, s_blk, s_p, s_half, s_c, s_t = yb.strides
    v = np.lib.stride_tricks.as_strided(
        yb,
        shape=(D, D, 2, NSP, NB, GB, N_WT, TW),
        strides=(NW * s_c, s_c, s_half, s_sp, s_blk,
                 TW * s_p + NW * s_c, s_t, s_p + s_c),
    )
    return np.ascontiguousarray(v).reshape(D * D, H, W)


def kernel(x1, x2):
    _install_axon_trace_shim()
    nc = _get_nc()
    in_maps = _prep_inputs(x1, x2)
    res = run_bass_kernel_spmd(nc, in_maps, core_ids=list(range(B)))
    kernel.last_results = res
    out = np.empty((B, D * D, H, W), dtype=np.float32)
    for b in range(B):
        yb = np.asarray(res.results[b]["y"]).astype(np.float32)
        out[b] = _deskew(yb)
    return out


# revision 12
# speedup vs baseline: 1.1652x; 1.1652x over previous
"""Correlation (FlowNet-style, max_displacement=4) on 8 TRN2 NeuronCores.

Full inputs x1, x2: [B=8, C=64, H=192, W=192] fp32. Output: [8, 81, 192, 192] fp32.
out[b, di*9+dj, h, w] = mean_c x1[b,c,h,w] * x2pad[b,c,h+di,w+dj]   (di,dj in [0,9))

Strategy: batch-parallel (1 batch per core). Per core the correlation is computed
as a banded Gram matrix on the TensorEngine: for each 16x8 (h,w) output tile,
one bf16 matmul with lhsT = x1 tile [K=64 channels, M=128 pixels] and rhs = padded
x2 window [64, 24*16=384] produces all 81 displacement dot products of every tile
pixel inside a skewed band of the 128x384 PSUM result. PSUM is evicted
(fp32->bf16) to SBUF by DVE/ACT in two-tile ops. The band is shipped in
4-dh-group blocks: partitions [32b, 32b+32) x band-column union [64b, 64b+192)
x both halves — 64 descriptors of 9216 B per DMA, which sprays across all 16
SDMA engines (8/16-descriptor DMAs only ever land on engines 0-7) at good
per-descriptor efficiency, for 1.33x byte inflation over the exact band
parallelogram (2.37x the useful output vs 4.74x if the whole PSUM band were
shipped). Only 4 out-DMAs per strip (24 total), alternating between the two
HWDGE rings (sync/scalar), so sequencer descriptor-gen (~640+40*ndesc ns per
DMA) stays off the critical path. The band is deskewed on the host with a
zero-copy strided view. x1 is pre-scaled by 1/64 on the host (exact) so the
matmul output is directly the channel mean.

The h axis is split into two halves living on partitions 0-63 / 64-127 (K=64
each), interleaved so paired matmuls run concurrently on disjoint PE
row-groups. Inputs are loaded in three h-chunks (separate tiles, small first
chunk) interleaved with compute so the PE starts early.
"""

import sys
import types

import numpy as np
import ml_dtypes

import concourse.bacc as bacc
from concourse import mybir
from concourse.tile import TileContext
from concourse.bass_utils import run_bass_kernel_spmd

B, C, H, W = 8, 64, 192, 192
MAXD = 4
D = 2 * MAXD + 1  # 9
HP, WP = H + 2 * MAXD, W + 2 * MAXD  # 200, 200

TH, TW = 16, 8            # output tile (h, w) -> M = 128
NH, NW = TH + 2 * MAXD, TW + 2 * MAXD  # x2 window 24 x 16 -> N = 384
NSP = H // (2 * TH)       # 6 strips per partition-half
N_WT = W // TW            # 24 w-tiles
HHALF = H // 2            # 96 rows per partition-half
SLAB = HHALF + 2 * MAXD   # 104 padded x2 rows per half
BCOL = D * NW             # 144 band columns per dh-group
GB = 4                    # dh-groups per out-DMA block
NB = TH // GB             # 4 blocks per strip
BW = BCOL + (GB - 1) * NW  # 192 block band columns

# Input h-chunking: strip ranges per chunk and the x2 slab rows they need.
X1_CHUNKS = [(0, 1), (1, 3), (3, 6)]              # strip ranges
X2_CHUNKS = [(0, 24), (16, 56), (48, 104)]        # x2 local row ranges

BF16 = ml_dtypes.bfloat16


def _install_axon_trace_shim():
    """The image's antenv package lacks axon_hooks; run_bass_kernel_spmd
    crashes on import when trace=True. Provide the hook from the boot module
    so tracing works instead of raising."""
    if "antenv.axon_hooks" in sys.modules:
        return
    try:
        import trn_agent_boot.trn_boot as tb

        hook = tb._ntff_profile_via_ctypes("/opt/axon/libaxon_pjrt.so")
    except Exception:
        hook = None
    mod = types.ModuleType("antenv.axon_hooks")
    mod.get_axon_ntff_profile_hook = lambda: hook
    mod.set_axon_ntff_profile_hook = lambda h: None
    sys.modules["antenv.axon_hooks"] = mod


def build_nc():
    nc = bacc.Bacc("TRN2", target_bir_lowering=False, debug=False)
    # x1 arrives pre-tiled: [128, strip, wtile, 128 pixels] — walrus requires
    # the matmul weights AP to have a single free dimension.
    x1s = nc.dram_tensor("x1s", [128, NSP, N_WT, TH * TW], mybir.dt.bfloat16, kind="ExternalInput")
    x2s = nc.dram_tensor("x2s", [128, SLAB, WP], mybir.dt.bfloat16, kind="ExternalInput")
    y = nc.dram_tensor("y", [NSP, 2, NB, GB * TW, 2, BW, N_WT // 2], mybir.dt.bfloat16, kind="ExternalOutput")

    with TileContext(nc) as tc:
        with (
            tc.tile_pool(name="imgs", bufs=1) as imgs,
            tc.tile_pool(name="outs", bufs=3) as outs,
            tc.tile_pool(name="psum", bufs=4, space="PSUM") as psum,
        ):
            # Chunked input tiles (separate tiles -> precise chunk->matmul deps).
            x1c, x2c = [], []
            for ci in range(3):
                s0, s1 = X1_CHUNKS[ci]
                r0, r1 = X2_CHUNKS[ci]
                x2t = imgs.tile([128, r1 - r0, WP], mybir.dt.bfloat16, tag=f"x2c{ci}")
                nc.sync.dma_start(out=x2t[:], in_=x2s[:, r0:r1, :])
                x1t = imgs.tile([128, s1 - s0, N_WT, TH * TW], mybir.dt.bfloat16, tag=f"x1c{ci}")
                nc.scalar.dma_start(out=x1t[:], in_=x1s[:, s0:s1])
                x2c.append(x2t)
                x1c.append(x1t)

            copy_k = 0
            for sp in range(NSP):
                ci = next(i for i, (s0, s1) in enumerate(X1_CHUNKS) if s0 <= sp < s1)
                hl = sp * TH - X2_CHUNKS[ci][0]   # row offset within x2 chunk
                spl = sp - X1_CHUNKS[ci][0]       # strip offset within x1 chunk
                # Both halves in one tile; [col, t] minor so each band block
                # is one contiguous run per (partition, half). Two w-half
                # tiles per strip: the first half's blocks ship while the
                # second half is still evicting, halving the drain tail.
                ybufs = [outs.tile([128, 2, NH * NW, N_WT // 2], mybir.dt.bfloat16,
                                   name=f"ybuf{ws}_{sp}", tag=f"ybuf{ws}")
                         for ws in range(2)]
                for tp in range(N_WT // 2):       # pairs of w-tiles
                    # Interleave the two partition halves so adjacent matmuls
                    # sit on disjoint PE row-groups and execute concurrently.
                    for half in range(2):
                        p0 = 64 * half
                        pt = psum.tile([128, 1024], mybir.dt.float32)
                        for u in range(2):
                            t = 2 * tp + u
                            w0 = t * TW
                            nc.tensor.matmul(
                                pt[:, 512 * u:512 * u + NH * NW],
                                lhsT=x1c[ci][p0:p0 + 64, spl, t, :],
                                rhs=x2c[ci][p0:p0 + 64, hl:hl + NH, w0:w0 + NW],
                                start=True, stop=True,
                            )
                        # Evict both tiles with one op; alternate DVE / ACT.
                        src = pt[:].rearrange("p (a b) -> p b a", a=2)[:, 0:NH * NW, :]
                        ws, tpl = divmod(tp, N_WT // 4)
                        dst = ybufs[ws][:, half, :, 2 * tpl:2 * tpl + 2]
                        if copy_k % 2 == 0:
                            nc.vector.tensor_copy(dst, src)
                        else:
                            nc.scalar.copy(dst, src)
                        copy_k += 1
                    if tp == N_WT // 4 - 1:
                        # First w-half complete: ship its blocks now, while
                        # the second w-half is still computing/evicting.
                        for blk in range(NB):
                            eng = nc.sync if (sp + blk) % 2 == 0 else nc.scalar
                            eng.dma_start(
                                out=y[sp, 0, blk],
                                in_=ybufs[0][32 * blk:32 * blk + 32, :,
                                             NW * GB * blk:NW * GB * blk + BW, :],
                            )
                # Second w-half blocks: partitions [32b, 32b+32), column
                # union [64b, 64b+192), both halves: 64 descriptors x 4608 B.
                for blk in range(NB):
                    eng = nc.sync if (sp + blk) % 2 == 1 else nc.scalar
                    eng.dma_start(
                        out=y[sp, 1, blk],
                        in_=ybufs[1][32 * blk:32 * blk + 32, :,
                                     NW * GB * blk:NW * GB * blk + BW, :],
                    )

    nc.compile()
    return nc


_NC_CACHE = None


def _get_nc():
    global _NC_CACHE
    if _NC_CACHE is None:
        _NC_CACHE = build_nc()
    return _NC_CACHE


def _prep_inputs(x1, x2):
    """Host-side shard prep: scale, pad, split h into partition halves, bf16."""
    in_maps = []
    x1 = np.asarray(x1, dtype=np.float32)
    x2 = np.asarray(x2, dtype=np.float32)
    x1h = (x1 * (1.0 / C)).astype(BF16)
    x2h = x2.astype(BF16)
    for b in range(B):
        # x1: [64, 192, 192] -> pre-tiled [128 = half*64+c, sp, t, dh*TW+dw]
        a = x1h[b].reshape(C, 2, NSP, TH, N_WT, TW)
        a = a.transpose(1, 0, 2, 4, 3, 5).reshape(128, NSP, N_WT, TH * TW)
        # x2: pad to [64, 200, 200], two overlapping 104-row slabs
        p = np.zeros((C, HP, WP), dtype=BF16)
        p[:, MAXD:MAXD + H, MAXD:MAXD + W] = x2h[b]
        s = np.stack([p[:, 0:SLAB, :], p[:, HHALF:HHALF + SLAB, :]], axis=0)
        s = s.reshape(2 * C, SLAB, WP)
        in_maps.append({"x1s": np.ascontiguousarray(a), "x2s": np.ascontiguousarray(s)})
    return in_maps


def _deskew(yb):
    """yb: [NSP, 2, NB, GB*TW, 2, BW, N_WT//2] fp32 (one batch) -> [81, 192, 192].

    h = half*96 + sp*TH + 4*blk + gin,  w = (ws*12 + t)*TW + dw; the value for
    displacement (di, dj) at (gin, dw) sits at block column
    16*gin + 16*di + dw + dj of partition gin*8+dw.
    """
    s_sp, s_ws, s_blk, s_p, s_half, s_c, s_t = yb.strides
    v = np.lib.stride_tricks.as_strided(
        yb,
        shape=(D, D, 2, NSP, NB, GB, 2, N_WT // 2, TW),
        strides=(NW * s_c, s_c, s_half, s_sp, s_blk,
                 TW * s_p + NW * s_c, s_ws, s_t, s_p + s_c),
    )
    return np.ascontiguousarray(v).reshape(D * D, H, W)


def kernel(x1, x2):
    _install_axon_trace_shim()
    nc = _get_nc()
    in_maps = _prep_inputs(x1, x2)
    res = run_bass_kernel_spmd(nc, in_maps, core_ids=list(range(B)))
    kernel.last_results = res
    out = np.empty((B, D * D, H, W), dtype=np.float32)
    for b in range(B):
        yb = np.asarray(res.results[b]["y"]).astype(np.float32)
        out[b] = _deskew(yb)
    return out


# revision 13
# speedup vs baseline: 1.2226x; 1.0493x over previous
"""Correlation (FlowNet-style, max_displacement=4) on 8 TRN2 NeuronCores.

Full inputs x1, x2: [B=8, C=64, H=192, W=192] fp32. Output: [8, 81, 192, 192] fp32.
out[b, di*9+dj, h, w] = mean_c x1[b,c,h,w] * x2pad[b,c,h+di,w+dj]   (di,dj in [0,9))

Strategy: batch-parallel (1 batch per core). Per core the correlation is computed
as a banded Gram matrix on the TensorEngine: for each 16x8 (h,w) output tile,
one bf16 matmul with lhsT = x1 tile [K=64 channels, M=128 pixels] and rhs = padded
x2 window [64, 24*16=384] produces all 81 displacement dot products of every tile
pixel inside a skewed band of the 128x384 PSUM result. PSUM is evicted
(fp32->bf16) to SBUF by DVE/ACT in two-tile ops. The band is shipped in
4-dh-group blocks: partitions [32b, 32b+32) x band-column union [64b, 64b+192)
x both halves — 64 descriptors of 9216 B per DMA, which sprays across all 16
SDMA engines (8/16-descriptor DMAs only ever land on engines 0-7) at good
per-descriptor efficiency, for 1.33x byte inflation over the exact band
parallelogram (2.37x the useful output vs 4.74x if the whole PSUM band were
shipped). Only 4 out-DMAs per strip (24 total), alternating between the two
HWDGE rings (sync/scalar), so sequencer descriptor-gen (~640+40*ndesc ns per
DMA) stays off the critical path. The band is deskewed on the host with a
zero-copy strided view. x1 is pre-scaled by 1/64 on the host (exact) so the
matmul output is directly the channel mean.

The h axis is split into two halves living on partitions 0-63 / 64-127 (K=64
each), interleaved so paired matmuls run concurrently on disjoint PE
row-groups. Inputs are loaded in three h-chunks (separate tiles, small first
chunk) interleaved with compute so the PE starts early.
"""

import sys
import types

import numpy as np
import ml_dtypes

import concourse.bacc as bacc
from concourse import mybir
from concourse.tile import TileContext
from concourse.bass_utils import run_bass_kernel_spmd

B, C, H, W = 8, 64, 192, 192
MAXD = 4
D = 2 * MAXD + 1  # 9
HP, WP = H + 2 * MAXD, W + 2 * MAXD  # 200, 200

TH, TW = 16, 8            # output tile (h, w) -> M = 128
NH, NW = TH + 2 * MAXD, TW + 2 * MAXD  # x2 window 24 x 16 -> N = 384
NSP = H // (2 * TH)       # 6 strips per partition-half
N_WT = W // TW            # 24 w-tiles
HHALF = H // 2            # 96 rows per partition-half
SLAB = HHALF + 2 * MAXD   # 104 padded x2 rows per half
BCOL = D * NW             # 144 band columns per dh-group
GB = 4                    # dh-groups per out-DMA block
NB = TH // GB             # 4 blocks per strip
BW = BCOL + (GB - 1) * NW  # 192 block band columns

# Input h-chunking: strip ranges per chunk and the x2 slab rows they need.
X1_CHUNKS = [(0, 1), (1, 3), (3, 6)]              # strip ranges
X2_CHUNKS = [(0, 24), (16, 56), (48, 104)]        # x2 local row ranges

BF16 = ml_dtypes.bfloat16


def _install_axon_trace_shim():
    """The image's antenv package lacks axon_hooks; run_bass_kernel_spmd
    crashes on import when trace=True. Provide the hook from the boot module
    so tracing works instead of raising."""
    if "antenv.axon_hooks" in sys.modules:
        return
    try:
        import trn_agent_boot.trn_boot as tb

        hook = tb._ntff_profile_via_ctypes("/opt/axon/libaxon_pjrt.so")
    except Exception:
        hook = None
    mod = types.ModuleType("antenv.axon_hooks")
    mod.get_axon_ntff_profile_hook = lambda: hook
    mod.set_axon_ntff_profile_hook = lambda h: None
    sys.modules["antenv.axon_hooks"] = mod


def build_nc():
    nc = bacc.Bacc("TRN2", target_bir_lowering=False, debug=False)
    # x1 arrives pre-tiled: [128, strip, wtile, 128 pixels] — walrus requires
    # the matmul weights AP to have a single free dimension.
    x1s = nc.dram_tensor("x1s", [128, NSP, N_WT, TH * TW], mybir.dt.bfloat16, kind="ExternalInput")
    x2s = nc.dram_tensor("x2s", [128, SLAB, WP], mybir.dt.bfloat16, kind="ExternalInput")
    y = nc.dram_tensor("y", [NSP, NB, GB * TW, 2, BW, N_WT], mybir.dt.bfloat16, kind="ExternalOutput")

    with TileContext(nc) as tc:
        with (
            tc.tile_pool(name="imgs", bufs=1) as imgs,
            tc.tile_pool(name="outs", bufs=3) as outs,
            tc.tile_pool(name="psum", bufs=4, space="PSUM") as psum,
        ):
            # Chunked input tiles (separate tiles -> precise chunk->matmul deps).
            x1c, x2c = [], []
            for ci in range(3):
                s0, s1 = X1_CHUNKS[ci]
                r0, r1 = X2_CHUNKS[ci]
                x2t = imgs.tile([128, r1 - r0, WP], mybir.dt.bfloat16, tag=f"x2c{ci}")
                nc.sync.dma_start(out=x2t[:], in_=x2s[:, r0:r1, :])
                x1t = imgs.tile([128, s1 - s0, N_WT, TH * TW], mybir.dt.bfloat16, tag=f"x1c{ci}")
                nc.scalar.dma_start(out=x1t[:], in_=x1s[:, s0:s1])
                x2c.append(x2t)
                x1c.append(x1t)

            copy_k = 0
            for sp in range(NSP):
                ci = next(i for i, (s0, s1) in enumerate(X1_CHUNKS) if s0 <= sp < s1)
                hl = sp * TH - X2_CHUNKS[ci][0]   # row offset within x2 chunk
                spl = sp - X1_CHUNKS[ci][0]       # strip offset within x1 chunk
                # Both halves in one tile; [col, t] minor so each band block
                # is one contiguous run per (partition, half).
                ybuf = outs.tile([128, 2, NH * NW, N_WT], mybir.dt.bfloat16,
                                 name=f"ybuf_{sp}", tag="ybuf")
                for tp in range(N_WT // 2):       # pairs of w-tiles
                    # Interleave the two partition halves so adjacent matmuls
                    # sit on disjoint PE row-groups and execute concurrently.
                    for half in range(2):
                        p0 = 64 * half
                        pt = psum.tile([128, 1024], mybir.dt.float32)
                        for u in range(2):
                            t = 2 * tp + u
                            w0 = t * TW
                            nc.tensor.matmul(
                                pt[:, 512 * u:512 * u + NH * NW],
                                lhsT=x1c[ci][p0:p0 + 64, spl, t, :],
                                rhs=x2c[ci][p0:p0 + 64, hl:hl + NH, w0:w0 + NW],
                                start=True, stop=True,
                            )
                        # Evict both tiles with one op; alternate DVE / ACT.
                        src = pt[:].rearrange("p (a b) -> p b a", a=2)[:, 0:NH * NW, :]
                        dst = ybuf[:, half, :, 2 * tp:2 * tp + 2]
                        if copy_k % 2 == 0:
                            nc.vector.tensor_copy(dst, src)
                        else:
                            nc.scalar.copy(dst, src)
                        copy_k += 1
                # Band out in 4-group blocks: partitions [32b, 32b+32), column
                # union [64b, 64b+192), both halves: 64 descriptors x 9216 B.
                for blk in range(NB):
                    eng = nc.sync if (sp + blk) % 2 == 0 else nc.scalar
                    eng.dma_start(
                        out=y[sp, blk],
                        in_=ybuf[32 * blk:32 * blk + 32, :,
                                 NW * GB * blk:NW * GB * blk + BW, :],
                    )

    nc.compile()
    return nc


_NC_CACHE = None


def _get_nc():
    global _NC_CACHE
    if _NC_CACHE is None:
        _NC_CACHE = build_nc()
    return _NC_CACHE


def _prep_inputs(x1, x2):
    """Host-side shard prep: scale, pad, split h into partition halves, bf16."""
    in_maps = []
    x1 = np.asarray(x1, dtype=np.float32)
    x2 = np.asarray(x2, dtype=np.float32)
    x1h = (x1 * (1.0 / C)).astype(BF16)
    x2h = x2.astype(BF16)
    for b in range(B):
        # x1: [64, 192, 192] -> pre-tiled [128 = half*64+c, sp, t, dh*TW+dw]
        a = x1h[b].reshape(C, 2, NSP, TH, N_WT, TW)
        a = a.transpose(1, 0, 2, 4, 3, 5).reshape(128, NSP, N_WT, TH * TW)
        # x2: pad to [64, 200, 200], two overlapping 104-row slabs
        p = np.zeros((C, HP, WP), dtype=BF16)
        p[:, MAXD:MAXD + H, MAXD:MAXD + W] = x2h[b]
        s = np.stack([p[:, 0:SLAB, :], p[:, HHALF:HHALF + SLAB, :]], axis=0)
        s = s.reshape(2 * C, SLAB, WP)
        in_maps.append({"x1s": np.ascontiguousarray(a), "x2s": np.ascontiguousarray(s)})
    return in_maps


def _deskew(yb):
    """yb: [NSP, NB, GB*TW, 2, BW, N_WT] fp32 (one batch) -> [81, 192, 192].

    h = half*96 + sp*TH + 4*blk + gin,  w = t*TW + dw; the value for
    displacement (di, dj) at (gin, dw) sits at block column
    16*gin + 16*di + dw + dj of partition gin*8+dw.
    """
    s_sp, s_blk, s_p, s_half, s_c, s_t = yb.strides
    v = np.lib.stride_tricks.as_strided(
        yb,
        shape=(D, D, 2, NSP, NB, GB, N_WT, TW),
        strides=(NW * s_c, s_c, s_half, s_sp, s_blk,
                 TW * s_p + NW * s_c, s_t, s_p + s_c),
    )
    return np.ascontiguousarray(v).reshape(D * D, H, W)


def kernel(x1, x2):
    _install_axon_trace_shim()
    nc = _get_nc()
    in_maps = _prep_inputs(x1, x2)
    res = run_bass_kernel_spmd(nc, in_maps, core_ids=list(range(B)))
    kernel.last_results = res
    out = np.empty((B, D * D, H, W), dtype=np.float32)
    for b in range(B):
        yb = np.asarray(res.results[b]["y"]).astype(np.float32)
        out[b] = _deskew(yb)
    return out


# revision 14
# speedup vs baseline: 1.2244x; 1.0015x over previous
"""Correlation (FlowNet-style, max_displacement=4) on 8 TRN2 NeuronCores.

Full inputs x1, x2: [B=8, C=64, H=192, W=192] fp32. Output: [8, 81, 192, 192] fp32.
out[b, di*9+dj, h, w] = mean_c x1[b,c,h,w] * x2pad[b,c,h+di,w+dj]   (di,dj in [0,9))

Strategy: batch-parallel (1 batch per core). Per core the correlation is computed
as a banded Gram matrix on the TensorEngine: for each 16x8 (h,w) output tile,
one bf16 matmul with lhsT = x1 tile [K=64 channels, M=128 pixels] and rhs = padded
x2 window [64, 24*16=384] produces all 81 displacement dot products of every tile
pixel inside a skewed band of the 128x384 PSUM result. PSUM is evicted
(fp32->bf16) to SBUF by DVE/ACT in two-tile ops. The band is shipped in
4-dh-group blocks: partitions [32b, 32b+32) x band-column union [64b, 64b+192)
x both halves — 64 descriptors of 9216 B per DMA, which sprays across all 16
SDMA engines (8/16-descriptor DMAs only ever land on engines 0-7) at good
per-descriptor efficiency, for 1.33x byte inflation over the exact band
parallelogram (2.37x the useful output vs 4.74x if the whole PSUM band were
shipped). Only 4 out-DMAs per strip (24 total), alternating between the two
HWDGE rings (sync/scalar), so sequencer descriptor-gen (~640+40*ndesc ns per
DMA) stays off the critical path. The band is deskewed on the host with a
zero-copy strided view. x1 is pre-scaled by 1/64 on the host (exact) so the
matmul output is directly the channel mean.

The h axis is split into two halves living on partitions 0-63 / 64-127 (K=64
each), interleaved so paired matmuls run concurrently on disjoint PE
row-groups. Inputs are loaded in three h-chunks (separate tiles, small first
chunk) interleaved with compute so the PE starts early.
"""

import sys
import types

import numpy as np
import ml_dtypes

import concourse.bacc as bacc
from concourse import mybir
from concourse.tile import TileContext
from concourse.bass_utils import run_bass_kernel_spmd

B, C, H, W = 8, 64, 192, 192
MAXD = 4
D = 2 * MAXD + 1  # 9
HP, WP = H + 2 * MAXD, W + 2 * MAXD  # 200, 200

TH, TW = 16, 8            # output tile (h, w) -> M = 128
NH, NW = TH + 2 * MAXD, TW + 2 * MAXD  # x2 window 24 x 16 -> N = 384
NSP = H // (2 * TH)       # 6 strips per partition-half
N_WT = W // TW            # 24 w-tiles
HHALF = H // 2            # 96 rows per partition-half
SLAB = HHALF + 2 * MAXD   # 104 padded x2 rows per half
BCOL = D * NW             # 144 band columns per dh-group
GB = 4                    # dh-groups per out-DMA block
NB = TH // GB             # 4 blocks per strip
BW = BCOL + (GB - 1) * NW  # 192 block band columns

# Input h-chunking: strip ranges per chunk and the x2 slab rows they need.
X1_CHUNKS = [(0, 1), (1, 3), (3, 6)]              # strip ranges
X2_CHUNKS = [(0, 24), (16, 56), (48, 104)]        # x2 local row ranges

BF16 = ml_dtypes.bfloat16


def _install_axon_trace_shim():
    """The image's antenv package lacks axon_hooks; run_bass_kernel_spmd
    crashes on import when trace=True. Provide the hook from the boot module
    so tracing works instead of raising."""
    if "antenv.axon_hooks" in sys.modules:
        return
    try:
        import trn_agent_boot.trn_boot as tb

        hook = tb._ntff_profile_via_ctypes("/opt/axon/libaxon_pjrt.so")
    except Exception:
        hook = None
    mod = types.ModuleType("antenv.axon_hooks")
    mod.get_axon_ntff_profile_hook = lambda: hook
    mod.set_axon_ntff_profile_hook = lambda h: None
    sys.modules["antenv.axon_hooks"] = mod


def build_nc():
    nc = bacc.Bacc("TRN2", target_bir_lowering=False, debug=False)
    # x1 arrives pre-tiled: [128, strip, wtile, 128 pixels] — walrus requires
    # the matmul weights AP to have a single free dimension.
    x1s = nc.dram_tensor("x1s", [128, NSP, N_WT, TH * TW], mybir.dt.bfloat16, kind="ExternalInput")
    x2s = nc.dram_tensor("x2s", [128, SLAB, WP], mybir.dt.bfloat16, kind="ExternalInput")
    y = nc.dram_tensor("y", [NSP, NB, GB * TW, 2, BW, N_WT], mybir.dt.bfloat16, kind="ExternalOutput")

    with TileContext(nc) as tc:
        with (
            tc.tile_pool(name="imgs", bufs=1) as imgs,
            tc.tile_pool(name="outs", bufs=3) as outs,
            tc.tile_pool(name="psum", bufs=4, space="PSUM") as psum,
        ):
            # Chunked input tiles (separate tiles -> precise chunk->matmul deps).
            # Chunk 0 (exactly strip 0) is split into per-half 64-partition
            # tiles: each queue generates only 64 descriptors (~3.2us vs
            # ~5.8us for 128) before half 0 of strip 0 can start. Half-0
            # matmuls then run without row-group pairing (PE has 2x slack).
            x1c, x2c = [None], [None]
            r0, r1 = X2_CHUNKS[0]
            x2c0 = [imgs.tile([64, r1 - r0, WP], mybir.dt.bfloat16,
                              name=f"x2c0{h}", tag=f"x2c0{h}") for h in range(2)]
            x1c0 = [imgs.tile([64, 1, N_WT, TH * TW], mybir.dt.bfloat16,
                              name=f"x1c0{h}", tag=f"x1c0{h}") for h in range(2)]
            nc.sync.dma_start(out=x2c0[0][:], in_=x2s[0:64, r0:r1, :])
            nc.scalar.dma_start(out=x1c0[0][:], in_=x1s[0:64, 0:1])
            nc.scalar.dma_start(out=x2c0[1][:], in_=x2s[64:128, r0:r1, :])
            nc.sync.dma_start(out=x1c0[1][:], in_=x1s[64:128, 0:1])
            for ci in range(1, 3):
                s0, s1 = X1_CHUNKS[ci]
                r0, r1 = X2_CHUNKS[ci]
                x2t = imgs.tile([128, r1 - r0, WP], mybir.dt.bfloat16, tag=f"x2c{ci}")
                nc.sync.dma_start(out=x2t[:], in_=x2s[:, r0:r1, :])
                x1t = imgs.tile([128, s1 - s0, N_WT, TH * TW], mybir.dt.bfloat16, tag=f"x1c{ci}")
                nc.scalar.dma_start(out=x1t[:], in_=x1s[:, s0:s1])
                x2c.append(x2t)
                x1c.append(x1t)

            copy_k = 0
            for sp in range(NSP):
                ci = next(i for i, (s0, s1) in enumerate(X1_CHUNKS) if s0 <= sp < s1)
                hl = sp * TH - X2_CHUNKS[ci][0]   # row offset within x2 chunk
                spl = sp - X1_CHUNKS[ci][0]       # strip offset within x1 chunk
                # Both halves in one tile; [col, t] minor so each band block
                # is one contiguous run per (partition, half).
                ybuf = outs.tile([128, 2, NH * NW, N_WT], mybir.dt.bfloat16,
                                 name=f"ybuf_{sp}", tag="ybuf")
                # Strip 0: half-major (each half's inputs arrive on
                # their own 64-partition tiles); other strips: half-minor so
                # paired matmuls hit disjoint PE row-groups concurrently.
                order = ([(h, tp) for h in range(2) for tp in range(N_WT // 2)]
                         if sp == 0 else
                         [(h, tp) for tp in range(N_WT // 2) for h in range(2)])
                for half, tp in order:
                    if True:
                        p0 = 64 * half
                        pt = psum.tile([128, 1024], mybir.dt.float32,
                                       name=f"pt_{sp}_{half}_{tp}", tag="pt")
                        for u in range(2):
                            t = 2 * tp + u
                            w0 = t * TW
                            if sp == 0:
                                lhsT = x1c0[half][:, 0, t, :]
                                rhs = x2c0[half][:, hl:hl + NH, w0:w0 + NW]
                            else:
                                lhsT = x1c[ci][p0:p0 + 64, spl, t, :]
                                rhs = x2c[ci][p0:p0 + 64, hl:hl + NH, w0:w0 + NW]
                            nc.tensor.matmul(
                                pt[:, 512 * u:512 * u + NH * NW],
                                lhsT=lhsT, rhs=rhs,
                                start=True, stop=True,
                            )
                        # Evict both tiles with one op; alternate DVE / ACT.
                        src = pt[:].rearrange("p (a b) -> p b a", a=2)[:, 0:NH * NW, :]
                        dst = ybuf[:, half, :, 2 * tp:2 * tp + 2]
                        if copy_k % 2 == 0:
                            nc.vector.tensor_copy(dst, src)
                        else:
                            nc.scalar.copy(dst, src)
                        copy_k += 1
                # Band out in 4-group blocks: partitions [32b, 32b+32), column
                # union [64b, 64b+192), both halves: 64 descriptors x 9216 B.
                for blk in range(NB):
                    eng = nc.sync if (sp + blk) % 2 == 0 else nc.scalar
                    eng.dma_start(
                        out=y[sp, blk],
                        in_=ybuf[32 * blk:32 * blk + 32, :,
                                 NW * GB * blk:NW * GB * blk + BW, :],
                    )

    nc.compile()
    return nc


_NC_CACHE = None


def _get_nc():
    global _NC_CACHE
    if _NC_CACHE is None:
        _NC_CACHE = build_nc()
    return _NC_CACHE


def _prep_inputs(x1, x2):
    """Host-side shard prep: scale, pad, split h into partition halves, bf16."""
    in_maps = []
    x1 = np.asarray(x1, dtype=np.float32)
    x2 = np.asarray(x2, dtype=np.float32)
    x1h = (x1 * (1.0 / C)).astype(BF16)
    x2h = x2.astype(BF16)
    for b in range(B):
        # x1: [64, 192, 192] -> pre-tiled [128 = half*64+c, sp, t, dh*TW+dw]
        a = x1h[b].reshape(C, 2, NSP, TH, N_WT, TW)
        a = a.transpose(1, 0, 2, 4, 3, 5).reshape(128, NSP, N_WT, TH * TW)
        # x2: pad to [64, 200, 200], two overlapping 104-row slabs
        p = np.zeros((C, HP, WP), dtype=BF16)
        p[:, MAXD:MAXD + H, MAXD:MAXD + W] = x2h[b]
        s = np.stack([p[:, 0:SLAB, :], p[:, HHALF:HHALF + SLAB, :]], axis=0)
        s = s.reshape(2 * C, SLAB, WP)
        in_maps.append({"x1s": np.ascontiguousarray(a), "x2s": np.ascontiguousarray(s)})
    return in_maps


def _deskew(yb):
    """yb: [NSP, NB, GB*TW, 2, BW, N_WT] fp32 (one batch) -> [81, 192, 192].

    h = half*96 + sp*TH + 4*blk + gin,  w = t*TW + dw; the value for
    displacement (di, dj) at (gin, dw) sits at block column
    16*gin + 16*di + dw + dj of partition gin*8+dw.
    """
    s_sp, s_blk, s_p, s_half, s_c, s_t = yb.strides
    v = np.lib.stride_tricks.as_strided(
        yb,
        shape=(D, D, 2, NSP, NB, GB, N_WT, TW),
        strides=(NW * s_c, s_c, s_half, s_sp, s_blk,
                 TW * s_p + NW * s_c, s_t, s_p + s_c),
    )
    return np.ascontiguousarray(v).reshape(D * D, H, W)


def kernel(x1, x2):
    _install_axon_trace_shim()
    nc = _get_nc()
    in_maps = _prep_inputs(x1, x2)
    res = run_bass_kernel_spmd(nc, in_maps, core_ids=list(range(B)))
    kernel.last_results = res
    out = np.empty((B, D * D, H, W), dtype=np.float32)
    for b in range(B):
        yb = np.asarray(res.results[b]["y"]).astype(np.float32)
        out[b] = _deskew(yb)
    return out
